# revision 2
# baseline (speedup 1.0000x reference)
"""Bass/Tile kernel v2 for the bidirectional LSTM (S=512, B=64, I=H=512).

Sharding: 8 cores, data-parallel over batch. Each core runs BOTH directions
on a batch slice of 8; the two directions form independent per-step
dependency chains that hide each other's semaphore latency.

Per core, per direction (B=8):
  Transposed gates: gatesT PSUM tile [128, 128], free order (T, b) with
  gate-major tiles [i0..i3 | f0..f3 | o0..o3 | g0..g3]; partition p +
  j-tile <-> hidden unit 128j+p.
  Per step: 1 identity matmul copies the xprojT slice into PSUM
  (start=True), then 64 bf16 W-stationary matmuls accumulate W_hh @ h.
  Elementwise chain (6 sem hops): ACT sigmoid(i,f,o) -> ACT tanh(g) ->
  DVE ig=i*g, fc=f*c, c'=ig+fc -> ACT tanh(c') -> Pool h=o*tanh(c')
  (f32 -> bf16 fused convert) written into the bf16 output ring, which is
  BOTH the next step's matmul rhs and the DMA-out source (host converts to
  f32). No per-step transposes, no per-step DMA.
  Phase 1 (interleaved, ~5 ops/step): xprojT = W_ih @ x^T + bias in bf16,
  W-stationary matmuls into PSUM, Pool copy+bias into an SBUF ring.
Output: ring of 4x32-step chunks [128, 32, 64] bf16, 16 DMAs total.
"""

import sys
if "/opt/trn_rl_repo" not in sys.path:
    sys.path.insert(0, "/opt/trn_rl_repo")
import numpy as np
import ml_dtypes

import concourse.bass as bass
import concourse.bacc as bacc
import concourse.mybir as mybir
import concourse.tile as tile

F32 = mybir.dt.float32
F32R = mybir.dt.float32r
BF16 = mybir.dt.bfloat16
AF = mybir.ActivationFunctionType
ALU = mybir.AluOpType
BF16NP = ml_dtypes.bfloat16

S, B, I, H = 512, 64, 512, 512
NC = 8
BC = 8                     # batch per core
NT = 16                    # gate-column tiles of 128
NK = 4                     # contraction k-tiles of 128
TOK_BLK = 256              # phase-1 block = 256 tokens = 32 steps
SPB = TOK_BLK // BC        # steps per block window = 32
NBLK = S * BC // TOK_BLK   # 16 blocks per direction
OUT_CHUNK = 32             # steps per output DMA chunk
DIRS = ("f", "b")

# gate-major tile order [i0..i3 | f0..f3 | o0..o3 | g0..g3];
# PyTorch W row order is i,f,g,o.
_GIDX = [0, 1, 3, 2]


def _gatecols(T):
    g = _GIDX[T // 4]
    j = T % 4
    return g * H + 128 * j + np.arange(128)


def prep_core_inputs(inpt, W_ih_f, W_hh_f, b_ih_f, b_hh_f,
                     W_ih_b, W_hh_b, b_ih_b, b_hh_b):
    x_f = np.ascontiguousarray(inpt, dtype=np.float32)        # [S, B, I]
    x_b = np.ascontiguousarray(inpt[::-1], dtype=np.float32)

    shared = {}
    for d, (Wih, Whh, bih, bhh) in (("f", (W_ih_f, W_hh_f, b_ih_f, b_hh_f)),
                                    ("b", (W_ih_b, W_hh_b, b_ih_b, b_hh_b))):
        Wih = np.asarray(Wih, np.float32).copy()
        Whh = np.asarray(Whh, np.float32).copy()
        bias = np.asarray(bih, np.float32) + np.asarray(bhh, np.float32)
        # tanh(g) is computed as 2*sigmoid(2g)-1: pre-scale the g-gate's
        # weights and bias by 2 so one sigmoid covers all four gates.
        Wih[2 * H:3 * H, :] *= 2.0
        Whh[2 * H:3 * H, :] *= 2.0
        bias[2 * H:3 * H] *= 2.0
        # slabs [128(p), 4(k), 16(T), 128(m)]: slab[p,k,T,m] = W[gc(T,m), 128k+p]
        wih = np.empty((128, NK, NT, 128), np.float32)
        whh = np.empty((128, NK, NT, 128), np.float32)
        biasT = np.empty((128, NT), np.float32)
        for T in range(NT):
            cols = _gatecols(T)
            biasT[:, T] = bias[cols]
            for k in range(NK):
                wih[:, k, T, :] = Wih[cols, 128 * k:128 * (k + 1)].T
                whh[:, k, T, :] = Whh[cols, 128 * k:128 * (k + 1)].T
        shared[f"WihT_{d}"] = wih.astype(BF16NP)
        shared[f"WhhT_{d}"] = whh.astype(BF16NP)
        # bias as rank-1 matmul lhsT rows: [1, 16(T), 128(m)]
        shared[f"bias1_{d}"] = biasT.T[None, :, :].astype(BF16NP)
    shared["ident"] = np.eye(128, dtype=np.float32)
    shared["ones1"] = np.ones((1, TOK_BLK), dtype=np.float32).astype(BF16NP)

    in_maps = []
    for c in range(NC):
        m = dict(shared)
        bs = slice(c * BC, (c + 1) * BC)
        for d, x in (("f", x_f), ("b", x_b)):
            xs = x[:, bs, :]                              # [S, 8, I]
            xT = np.ascontiguousarray(xs.reshape(S * BC, I).T)
            m[f"xT_{d}"] = xT.astype(BF16NP)              # [I, 4096]
        in_maps.append(m)
    return in_maps


def assemble_output(results):
    out = np.empty((S, B, 2 * H), dtype=np.float32)
    for c in range(NC):
        bs = slice(c * BC, (c + 1) * BC)
        slab = np.asarray(results[c]["out"], dtype=np.float32)  # [128,S,64]
        for d, off, lo in (("f", 0, 0), ("b", H, 32)):
            arr = slab[:, :, lo:lo + 32].reshape(128, S, NK, BC)  # [p,t,j,b]
            h = arr.transpose(1, 3, 2, 0).reshape(S, BC, H)       # 128j+p
            out[:, bs, off:off + H] = h
    return out


def build_nc(n_steps=S):
    nc = bacc.Bacc("TRN2", target_bir_lowering=False, debug=False)

    n_blk = (n_steps + SPB - 1) // SPB

    dram = {}
    for d in DIRS:
        dram[f"xT_{d}"] = nc.declare_dram_parameter(
            f"xT_{d}", [I, S * BC], BF16, isOutput=False)
        dram[f"WihT_{d}"] = nc.declare_dram_parameter(
            f"WihT_{d}", [128, NK, NT, 128], BF16, isOutput=False)
        dram[f"WhhT_{d}"] = nc.declare_dram_parameter(
            f"WhhT_{d}", [128, NK, NT, 128], BF16, isOutput=False)
        dram[f"bias1_{d}"] = nc.declare_dram_parameter(
            f"bias1_{d}", [1, NT, 128], BF16, isOutput=False)
    dram["ident"] = nc.declare_dram_parameter("ident", [128, 128], F32,
                                              isOutput=False)
    dram["ones1"] = nc.declare_dram_parameter("ones1", [1, TOK_BLK], BF16,
                                              isOutput=False)
    out_d = nc.declare_dram_parameter("out", [128, n_steps, 2 * NK * BC],
                                      BF16, isOutput=True)

    xTr = {d: dram[f"xT_{d}"].rearrange("(k p) t -> p k t", p=128)
           for d in DIRS}

    with tile.TileContext(nc) as tc:
        with (
            tc.tile_pool(name="weights", bufs=1) as wpool,
            tc.tile_pool(name="xin", bufs=1) as xinp,
            tc.tile_pool(name="xpp", bufs=1) as xppp,
            tc.tile_pool(name="p1ps", bufs=2, space="PSUM") as p1ps,
            tc.tile_pool(name="gps", bufs=2, space="PSUM") as gpsp,
            tc.tile_pool(name="state", bufs=1) as spool,
            tc.tile_pool(name="estage", bufs=2) as epool,
            tc.tile_pool(name="oring", bufs=1) as oring,
        ):
            WihT, WhhT, bias1 = {}, {}, {}
            for d in DIRS:
                WihT[d] = wpool.tile([128, NK, NT, 128], BF16,
                                     tag=f"wih{d}", name=f"wih{d}")
                WhhT[d] = wpool.tile([128, NK, NT, 128], BF16,
                                     tag=f"whh{d}", name=f"whh{d}")
                bias1[d] = wpool.tile([1, NT, 128], BF16,
                                      tag=f"bias1{d}", name=f"bias1{d}")
                nc.sync.dma_start(WihT[d][:, :, :, :],
                                  dram[f"WihT_{d}"][:, :, :, :])
                nc.sync.dma_start(WhhT[d][:, :, :, :],
                                  dram[f"WhhT_{d}"][:, :, :, :])
                nc.sync.dma_start(bias1[d][:, :, :],
                                  dram[f"bias1_{d}"][:, :, :])
            ident = wpool.tile([128, 128], F32, tag="ident")
            ones1 = wpool.tile([1, TOK_BLK], BF16, tag="ones1")
            nc.sync.dma_start(ident[:, :], dram["ident"][:, :])
            nc.sync.dma_start(ones1[:, :], dram["ones1"][:, :])

            xblk = {d: [xinp.tile([128, NK, TOK_BLK], BF16, tag=f"xb{d}{i}",
                                  name=f"xb{d}{i}") for i in range(3)]
                    for d in DIRS}
            xpp = {d: [xppp.tile([128, NT, TOK_BLK], F32, tag=f"xp{d}{i}",
                                 name=f"xp{d}{i}") for i in range(2)]
                   for d in DIRS}

            def load_xblk(d, blk):
                nc.sync.dma_start(
                    xblk[d][blk % 3][:, :, :],
                    xTr[d][:, :, blk * TOK_BLK:(blk + 1) * TOK_BLK])

            def p1_ops(d, blk):
                dst = xpp[d][blk % 2]
                src = xblk[d][blk % 3]
                for T in range(NT):
                    ps = p1ps.tile([128, TOK_BLK], F32, tag="p1ps",
                                   name=f"p1_{d}{blk}_{T}")
                    yield ("bmm", ps, d, T)
                    for k in range(NK):
                        yield ("mm", ps, d, T, k, src)
                    for q in range(2):
                        yield ("cp", ps, d, T, dst, q)

            def emit_p1(op):
                if op[0] == "bmm":
                    _, ps, d, T = op
                    nc.tensor.matmul(ps[:, :], bias1[d][:, T, :],
                                     ones1[:, :], start=True, stop=False)
                elif op[0] == "mm":
                    _, ps, d, T, k, src = op
                    nc.tensor.matmul(ps[:, :], WihT[d][:, k, T, :],
                                     src[:, k, :],
                                     start=False, stop=(k == NK - 1))
                else:
                    _, ps, d, T, dst, q = op
                    lo, hi = q * (TOK_BLK // 2), (q + 1) * (TOK_BLK // 2)
                    nc.vector.tensor_copy(
                        dst[:, T, lo:hi], ps[:, lo:hi])

            # ---- prologue ----------------------------------------------
            for d in DIRS:
                for blk in range(min(3, n_blk)):
                    load_xblk(d, blk)
            for d in DIRS:
                for op in p1_ops(d, 0):
                    emit_p1(op)

            cst = {d: [spool.tile([128, NK * BC], F32, tag=f"c{d}{j}",
                                  name=f"c{d}{j}") for j in range(2)]
                   for d in DIRS}
            z0 = spool.tile([128, 2 * NK * BC], BF16, tag="z0")
            nc.vector.memset(z0[:, :], 0.0)
            for d in DIRS:
                nc.vector.memset(cst[d][0][:, :], 0.0)

            oslots = [oring.tile([128, OUT_CHUNK, 2 * NK * BC], BF16,
                                 tag=f"os{i}", name=f"os{i}")
                      for i in range(4)]

            pending = []

            for t in range(n_steps):
                cur, nxt = t % 2, (t + 1) % 2
                w, sm = t // SPB, t % SPB
                if sm == 0:
                    for d in DIRS:
                        if w + 3 < n_blk:
                            load_xblk(d, w + 3)
                    if w + 1 < n_blk:
                        pending = [op for pair in zip(p1_ops("f", w + 1),
                                                      p1_ops("b", w + 1))
                                   for op in pair]

                # previous-step h (bf16) lives in the output ring
                if t == 0:
                    hprev = z0
                else:
                    hprev = oslots[((t - 1) // OUT_CHUNK) % 4][:, (t - 1) % OUT_CHUNK, :]
                oslot = oslots[(t // OUT_CHUNK) % 4]

                # Emit dir f's FULL chain, then dir b's — antiphase-
                # compatible FIFO order so the two chains hide each other.
                for di, d in enumerate(DIRS):
                    g = gpsp.tile([128, NT * BC], F32, tag=f"g{d}",
                                  name=f"g{d}{t % 2}")
                    g4 = g[:, :].rearrange("p (T b) -> p T b", b=BC)
                    xsl = xpp[d][w % 2][:, :, sm * BC:(sm + 1) * BC]
                    nc.tensor.matmul(g[:, :], ident[:, :], xsl,
                                     start=True, stop=False,
                                     skip_group_check=True)
                    off = 32 * di
                    for k in range(NK):
                        for T in range(NT):
                            nc.tensor.matmul(
                                g4[:, T, :], WhhT[d][:, k, T, :],
                                hprev[:, off + k * BC:off + (k + 1) * BC],
                                start=False, stop=(k == NK - 1),
                                skip_group_check=True)

                    s = epool.tile([128, NT * BC], F32, tag=f"sg{d}",
                                   name=f"sg{d}")
                    nc.scalar.activation(s[:, :], g[:, :], AF.Sigmoid)

                    ig = epool.tile([128, NK * BC], F32, tag=f"ig{d}",
                                    name=f"ig{d}")
                    fc = epool.tile([128, NK * BC], F32, tag=f"fc{d}",
                                    name=f"fc{d}")
                    tg = epool.tile([128, NK * BC], F32, tag=f"tg{d}",
                                    name=f"tg{d}")
                    nc.gpsimd.tensor_mul(fc[:, :], s[:, 32:64],
                                         cst[d][cur][:, :])
                    nc.vector.tensor_scalar(tg[:, :], s[:, 96:128],
                                            2.0, -1.0, ALU.mult, ALU.add)
                    nc.vector.tensor_mul(ig[:, :], s[:, 0:32], tg[:, :])
                    nc.vector.tensor_add(cst[d][nxt][:, :], ig[:, :],
                                         fc[:, :])
                    tc_t = epool.tile([128, NK * BC], F32, tag=f"tc{d}",
                                      name=f"tc{d}")
                    nc.scalar.activation(tc_t[:, :], cst[d][nxt][:, :],
                                         AF.Tanh)
                    nc.gpsimd.tensor_mul(
                        oslot[:, sm, 32 * di:32 * di + 32],
                        s[:, 64:96], tc_t[:, :])

                    for _ in range(4):
                        if pending:
                            emit_p1(pending.pop(0))
                if sm == SPB - 1:
                    while pending:
                        emit_p1(pending.pop(0))
                if t % OUT_CHUNK == OUT_CHUNK - 1:
                    q = t // OUT_CHUNK
                    nc.sync.dma_start(
                        out_d[:, q * OUT_CHUNK:(q + 1) * OUT_CHUNK, :],
                        oslots[q % 4][:, :, :])

    nc.compile()
    return nc


# ---------------------------------------------------------------------------
from concourse.bass_utils import run_bass_kernel_spmd

_NC_CACHE = {}


def _get_nc():
    if "nc" not in _NC_CACHE:
        _NC_CACHE["nc"] = build_nc(n_steps=S)
    return _NC_CACHE["nc"]


def kernel(**inputs):
    nc = _get_nc()
    in_maps = prep_core_inputs(**inputs)
    res = run_bass_kernel_spmd(nc, in_maps, list(range(NC)))
    return assemble_output(res.results)


# revision 3
# speedup vs baseline: 1.7848x; 1.7848x over previous
"""Bass/Tile kernel v2 for the bidirectional LSTM (S=512, B=64, I=H=512).

Sharding: 8 cores, data-parallel over batch. Each core runs BOTH directions
on a batch slice of 8; the two directions form independent per-step
dependency chains that hide each other's latency.

Per core, per direction (B=8):
  Transposed gates: gatesT PSUM tile [128, 128], free order (T, b) with
  gate-major tiles [i0..i3 | f0..f3 | o0..o3 | g0..g3]; partition p of
  j-tile <-> hidden unit 128j+p.
  Per step: 1 fp32 identity matmul copies the xprojT slice into PSUM
  (start=True), then 64 bf16 W-stationary matmuls [128x128] x [128x8]
  accumulate W_hh @ h (4 k-waves x 16 tiles).
  Elementwise chain (5 sem hops): ACT sigmoid over ALL 128 gate cols
  (tanh(g) is 2*sigmoid(2g)-1; g's weights/bias pre-scaled by 2 on host)
  -> DVE tg'=2*sg-1, ig=i*tg' (+ Pool fc=f*c) -> DVE c'=ig+fc ->
  ACT tanh(c') -> Pool h=o*tanh(c') (f32->bf16 fused convert) written into
  the bf16 output ring, which is BOTH the next step's matmul rhs and the
  DMA-out source (host converts to f32). No per-step transposes or DMA.
  Phase 1 (interleaved, ~7 ops/step): per 256-token block, 16 PSUM tiles
  [128,256] = bias rank-1 matmul + 4 bf16 W_ih-stationary matmuls; DVE
  copies PSUM -> f32 SBUF xprojT ring (Pool cannot access PSUM; DVE cannot
  produce f32r/bf16 -- hence fp32 xprojT + fp32 id-copy matmul).
Output: ring of 4x32-step chunks [128, 32, 64] bf16, 16 DMAs total.
Sim (TimelineSim cost model, core 0): ~1.40 ms vs 5.47 ms for the v1
baseline; device correctness rel_err ~5.5e-3 (bf16 W/h/x).
"""

import sys
if "/opt/trn_rl_repo" not in sys.path:
    sys.path.insert(0, "/opt/trn_rl_repo")
import numpy as np
import ml_dtypes

import concourse.bass as bass
import concourse.bacc as bacc
import concourse.mybir as mybir
import concourse.tile as tile

F32 = mybir.dt.float32
F32R = mybir.dt.float32r
BF16 = mybir.dt.bfloat16
AF = mybir.ActivationFunctionType
ALU = mybir.AluOpType
BF16NP = ml_dtypes.bfloat16

S, B, I, H = 512, 64, 512, 512
NC = 8
BC = 8                     # batch per core
NT = 16                    # gate-column tiles of 128
NK = 4                     # contraction k-tiles of 128
TOK_BLK = 256              # phase-1 block = 256 tokens = 32 steps
SPB = TOK_BLK // BC        # steps per block window = 32
NBLK = S * BC // TOK_BLK   # 16 blocks per direction
OUT_CHUNK = 32             # steps per output DMA chunk
DIRS = ("f", "b")

# gate-major tile order [i0..i3 | f0..f3 | o0..o3 | g0..g3];
# PyTorch W row order is i,f,g,o.
_GIDX = [0, 1, 3, 2]


def _gatecols(T):
    g = _GIDX[T // 4]
    j = T % 4
    return g * H + 128 * j + np.arange(128)


def prep_core_inputs(inpt, W_ih_f, W_hh_f, b_ih_f, b_hh_f,
                     W_ih_b, W_hh_b, b_ih_b, b_hh_b):
    x_f = np.ascontiguousarray(inpt, dtype=np.float32)        # [S, B, I]
    x_b = np.ascontiguousarray(inpt[::-1], dtype=np.float32)

    shared = {}
    for d, (Wih, Whh, bih, bhh) in (("f", (W_ih_f, W_hh_f, b_ih_f, b_hh_f)),
                                    ("b", (W_ih_b, W_hh_b, b_ih_b, b_hh_b))):
        Wih = np.asarray(Wih, np.float32).copy()
        Whh = np.asarray(Whh, np.float32).copy()
        bias = np.asarray(bih, np.float32) + np.asarray(bhh, np.float32)
        # tanh(g) is computed as 2*sigmoid(2g)-1: pre-scale the g-gate's
        # weights and bias by 2 so one sigmoid covers all four gates.
        Wih[2 * H:3 * H, :] *= 2.0
        Whh[2 * H:3 * H, :] *= 2.0
        bias[2 * H:3 * H] *= 2.0
        # slabs [128(p), 4(k), 16(T), 128(m)]: slab[p,k,T,m] = W[gc(T,m), 128k+p]
        wih = np.empty((128, NK, NT, 128), np.float32)
        whh = np.empty((128, NK, NT, 128), np.float32)
        biasT = np.empty((128, NT), np.float32)
        for T in range(NT):
            cols = _gatecols(T)
            biasT[:, T] = bias[cols]
            for k in range(NK):
                wih[:, k, T, :] = Wih[cols, 128 * k:128 * (k + 1)].T
                whh[:, k, T, :] = Whh[cols, 128 * k:128 * (k + 1)].T
        shared[f"WihT_{d}"] = wih.astype(BF16NP)
        shared[f"WhhT_{d}"] = whh.astype(BF16NP)
        # bias as rank-1 matmul lhsT rows: [1, 16(T), 128(m)]
        shared[f"bias1_{d}"] = biasT.T[None, :, :].astype(BF16NP)
    shared["ident"] = np.eye(128, dtype=np.float32)
    shared["ones1"] = np.ones((1, TOK_BLK), dtype=np.float32).astype(BF16NP)

    in_maps = []
    for c in range(NC):
        m = dict(shared)
        bs = slice(c * BC, (c + 1) * BC)
        for d, x in (("f", x_f), ("b", x_b)):
            xs = x[:, bs, :]                              # [S, 8, I]
            xT = np.ascontiguousarray(xs.reshape(S * BC, I).T)
            m[f"xT_{d}"] = xT.astype(BF16NP)              # [I, 4096]
        in_maps.append(m)
    return in_maps


def assemble_output(results):
    out = np.empty((S, B, 2 * H), dtype=np.float32)
    for c in range(NC):
        bs = slice(c * BC, (c + 1) * BC)
        slab = np.asarray(results[c]["out"], dtype=np.float32)  # [128,S,64]
        for d, off, lo in (("f", 0, 0), ("b", H, 32)):
            arr = slab[:, :, lo:lo + 32].reshape(128, S, NK, BC)  # [p,t,j,b]
            h = arr.transpose(1, 3, 2, 0).reshape(S, BC, H)       # 128j+p
            out[:, bs, off:off + H] = h
    return out


def build_nc(n_steps=S):
    nc = bacc.Bacc("TRN2", target_bir_lowering=False, debug=False)

    n_blk = (n_steps + SPB - 1) // SPB

    dram = {}
    for d in DIRS:
        dram[f"xT_{d}"] = nc.declare_dram_parameter(
            f"xT_{d}", [I, S * BC], BF16, isOutput=False)
        dram[f"WihT_{d}"] = nc.declare_dram_parameter(
            f"WihT_{d}", [128, NK, NT, 128], BF16, isOutput=False)
        dram[f"WhhT_{d}"] = nc.declare_dram_parameter(
            f"WhhT_{d}", [128, NK, NT, 128], BF16, isOutput=False)
        dram[f"bias1_{d}"] = nc.declare_dram_parameter(
            f"bias1_{d}", [1, NT, 128], BF16, isOutput=False)
    dram["ident"] = nc.declare_dram_parameter("ident", [128, 128], F32,
                                              isOutput=False)
    dram["ones1"] = nc.declare_dram_parameter("ones1", [1, TOK_BLK], BF16,
                                              isOutput=False)
    out_d = nc.declare_dram_parameter("out", [128, n_steps, 2 * NK * BC],
                                      BF16, isOutput=True)

    xTr = {d: dram[f"xT_{d}"].rearrange("(k p) t -> p k t", p=128)
           for d in DIRS}

    with tile.TileContext(nc) as tc:
        with (
            tc.tile_pool(name="weights", bufs=1) as wpool,
            tc.tile_pool(name="xin", bufs=1) as xinp,
            tc.tile_pool(name="xpp", bufs=1) as xppp,
            tc.tile_pool(name="p1ps", bufs=2, space="PSUM") as p1ps,
            tc.tile_pool(name="gps", bufs=2, space="PSUM") as gpsp,
            tc.tile_pool(name="state", bufs=1) as spool,
            tc.tile_pool(name="estage", bufs=2) as epool,
            tc.tile_pool(name="oring", bufs=1) as oring,
        ):
            WihT, WhhT, bias1 = {}, {}, {}
            for d in DIRS:
                WihT[d] = wpool.tile([128, NK, NT, 128], BF16,
                                     tag=f"wih{d}", name=f"wih{d}")
                WhhT[d] = wpool.tile([128, NK, NT, 128], BF16,
                                     tag=f"whh{d}", name=f"whh{d}")
                bias1[d] = wpool.tile([1, NT, 128], BF16,
                                      tag=f"bias1{d}", name=f"bias1{d}")
                nc.sync.dma_start(WihT[d][:, :, :, :],
                                  dram[f"WihT_{d}"][:, :, :, :])
                nc.sync.dma_start(WhhT[d][:, :, :, :],
                                  dram[f"WhhT_{d}"][:, :, :, :])
                nc.sync.dma_start(bias1[d][:, :, :],
                                  dram[f"bias1_{d}"][:, :, :])
            ident = wpool.tile([128, 128], F32, tag="ident")
            ones1 = wpool.tile([1, TOK_BLK], BF16, tag="ones1")
            nc.sync.dma_start(ident[:, :], dram["ident"][:, :])
            nc.sync.dma_start(ones1[:, :], dram["ones1"][:, :])

            xblk = {d: [xinp.tile([128, NK, TOK_BLK], BF16, tag=f"xb{d}{i}",
                                  name=f"xb{d}{i}") for i in range(3)]
                    for d in DIRS}
            xpp = {d: [xppp.tile([128, NT, TOK_BLK], F32, tag=f"xp{d}{i}",
                                 name=f"xp{d}{i}") for i in range(2)]
                   for d in DIRS}

            def load_xblk(d, blk):
                nc.sync.dma_start(
                    xblk[d][blk % 3][:, :, :],
                    xTr[d][:, :, blk * TOK_BLK:(blk + 1) * TOK_BLK])

            def p1_ops(d, blk):
                dst = xpp[d][blk % 2]
                src = xblk[d][blk % 3]
                for T in range(NT):
                    ps = p1ps.tile([128, TOK_BLK], F32, tag="p1ps",
                                   name=f"p1_{d}{blk}_{T}")
                    yield ("bmm", ps, d, T)
                    for k in range(NK):
                        yield ("mm", ps, d, T, k, src)
                    for q in range(2):
                        yield ("cp", ps, d, T, dst, q)

            def emit_p1(op):
                if op[0] == "bmm":
                    _, ps, d, T = op
                    nc.tensor.matmul(ps[:, :], bias1[d][:, T, :],
                                     ones1[:, :], start=True, stop=False)
                elif op[0] == "mm":
                    _, ps, d, T, k, src = op
                    nc.tensor.matmul(ps[:, :], WihT[d][:, k, T, :],
                                     src[:, k, :],
                                     start=False, stop=(k == NK - 1))
                else:
                    _, ps, d, T, dst, q = op
                    lo, hi = q * (TOK_BLK // 2), (q + 1) * (TOK_BLK // 2)
                    nc.vector.tensor_copy(
                        dst[:, T, lo:hi], ps[:, lo:hi])

            # ---- prologue ----------------------------------------------
            for d in DIRS:
                for blk in range(min(3, n_blk)):
                    load_xblk(d, blk)
            for d in DIRS:
                for op in p1_ops(d, 0):
                    emit_p1(op)

            cst = {d: [spool.tile([128, NK * BC], F32, tag=f"c{d}{j}",
                                  name=f"c{d}{j}") for j in range(2)]
                   for d in DIRS}
            z0 = spool.tile([128, 2 * NK * BC], BF16, tag="z0")
            nc.vector.memset(z0[:, :], 0.0)
            for d in DIRS:
                nc.vector.memset(cst[d][0][:, :], 0.0)

            oslots = [oring.tile([128, OUT_CHUNK, 2 * NK * BC], BF16,
                                 tag=f"os{i}", name=f"os{i}")
                      for i in range(4)]

            pending = []

            for t in range(n_steps):
                cur, nxt = t % 2, (t + 1) % 2
                w, sm = t // SPB, t % SPB
                if sm == 0:
                    for d in DIRS:
                        if w + 3 < n_blk:
                            load_xblk(d, w + 3)
                    if w + 1 < n_blk:
                        pending = [op for pair in zip(p1_ops("f", w + 1),
                                                      p1_ops("b", w + 1))
                                   for op in pair]

                # previous-step h (bf16) lives in the output ring
                if t == 0:
                    hprev = z0
                else:
                    hprev = oslots[((t - 1) // OUT_CHUNK) % 4][:, (t - 1) % OUT_CHUNK, :]
                oslot = oslots[(t // OUT_CHUNK) % 4]

                # Emit dir f's FULL chain, then dir b's — antiphase-
                # compatible FIFO order so the two chains hide each other.
                for di, d in enumerate(DIRS):
                    g = gpsp.tile([128, NT * BC], F32, tag=f"g{d}",
                                  name=f"g{d}{t % 2}")
                    g4 = g[:, :].rearrange("p (T b) -> p T b", b=BC)
                    xsl = xpp[d][w % 2][:, :, sm * BC:(sm + 1) * BC]
                    nc.tensor.matmul(g[:, :], ident[:, :], xsl,
                                     start=True, stop=False,
                                     skip_group_check=True)
                    off = 32 * di
                    for k in range(NK):
                        for T in range(NT):
                            nc.tensor.matmul(
                                g4[:, T, :], WhhT[d][:, k, T, :],
                                hprev[:, off + k * BC:off + (k + 1) * BC],
                                start=False, stop=(k == NK - 1),
                                skip_group_check=True)

                    s = epool.tile([128, NT * BC], F32, tag=f"sg{d}",
                                   name=f"sg{d}")
                    nc.scalar.activation(s[:, :], g[:, :], AF.Sigmoid)

                    ig = epool.tile([128, NK * BC], F32, tag=f"ig{d}",
                                    name=f"ig{d}")
                    fc = epool.tile([128, NK * BC], F32, tag=f"fc{d}",
                                    name=f"fc{d}")
                    tg = epool.tile([128, NK * BC], F32, tag=f"tg{d}",
                                    name=f"tg{d}")
                    nc.gpsimd.tensor_mul(fc[:, :], s[:, 32:64],
                                         cst[d][cur][:, :])
                    nc.vector.tensor_scalar(tg[:, :], s[:, 96:128],
                                            2.0, -1.0, ALU.mult, ALU.add)
                    nc.vector.tensor_mul(ig[:, :], s[:, 0:32], tg[:, :])
                    nc.vector.tensor_add(cst[d][nxt][:, :], ig[:, :],
                                         fc[:, :])
                    tc_t = epool.tile([128, NK * BC], F32, tag=f"tc{d}",
                                      name=f"tc{d}")
                    nc.scalar.activation(tc_t[:, :], cst[d][nxt][:, :],
                                         AF.Tanh)
                    nc.gpsimd.tensor_mul(
                        oslot[:, sm, 32 * di:32 * di + 32],
                        s[:, 64:96], tc_t[:, :])

                for _ in range(8):
                    if pending:
                        emit_p1(pending.pop(0))
                if sm == SPB - 1:
                    while pending:
                        emit_p1(pending.pop(0))
                if t % OUT_CHUNK == OUT_CHUNK - 1:
                    q = t // OUT_CHUNK
                    nc.sync.dma_start(
                        out_d[:, q * OUT_CHUNK:(q + 1) * OUT_CHUNK, :],
                        oslots[q % 4][:, :, :])

    nc.compile()
    return nc


# ---------------------------------------------------------------------------
from concourse.bass_utils import run_bass_kernel_spmd

_NC_CACHE = {}


def _get_nc():
    if "nc" not in _NC_CACHE:
        _NC_CACHE["nc"] = build_nc(n_steps=S)
    return _NC_CACHE["nc"]


def kernel(**inputs):
    nc = _get_nc()
    in_maps = prep_core_inputs(**inputs)
    res = run_bass_kernel_spmd(nc, in_maps, list(range(NC)))
    return assemble_output(res.results)
